# revision 20
# baseline (speedup 1.0000x reference)
"""Trainium2 Bass kernel for HierarchicalBG (bicubic pano sampling + MLP).

Strategy (data-parallel over rays, 8 cores):
- Level-2 grid weight is identically 0 (mip clipped to [1,3]) -> never read.
- Rays classified by mip: A (mip<2: needs level0+level1), B (needs l0 only),
  C (mip>=3: output exactly 0, not shipped to device).
- Tables are "footprint-slot" fp16: slot (R,C) = the full 4x4x8 bicubic
  footprint (256B) -> one dma_gather descriptor per (ray, level).
- x-range strip decomposition + LPT bin-packing of strips to cores keeps
  per-instruction idx ranges within int16 for dma_gather; static per-position
  capacities (CAPA/CAPB) with fabricated zero-weight pad rays.
- Device: fp16 weighting multiply (2x DVE mode) + k-tap tree; j-taps and the
  8->128 MLP are fused into one PE matmul per ray column via block-diag
  weights; relu on ACT; 128->3 on PE (col-tiled into one PSUM bank).
"""

import numpy as np
from contextlib import ExitStack

PI = float(np.pi)
P = 128
NCORES = 8

# static per-position ray capacities (slots), multiples of 128
CAPA = [1664, 1152, 1024, 896, 768, 640, 512, 384, 128, 128]   # level-1 strips
CAPB = [4224, 4224, 4096, 3072, 2688, 2176, 1280, 1664]        # l0-only slabs
PIECE_A, PIECE_B = 1536, 4096
FA = sum(CAPA) // 128            # 57 cols
FB = sum(CAPB) // 128            # 183 cols
FL0 = FA + FB                    # 240 cols through l0 + MLP
F = 256                          # total slot cols per core
MAXCHUNK = 1024                  # dma_gather per-instruction idx limit

W0, H0 = 1024, 512
W1_, H1 = 2048, 1024
SW_A1 = 31                       # l1 strip width (cols); idx = R*31+c <= 31774
SW_A0 = 20                       # l0 window width for one l1 strip
SW_B = 21                        # l0 mini-slab width
NSLOT_A1 = (H1 + 1) * SW_A1      # 31775
NSLOT_A0 = (H0 + 1) * SW_A0      # 10260
NSLOT_B = (H0 + 1) * SW_B        # 10773
NPA, NPB = len(CAPA), len(CAPB)

saTexel = 4.0 * PI / (6.0 * 2048 * 2048)
MIPC1 = 1.0 / (2.0 * np.log(2.0))
MIPC2 = -float(np.log(saTexel)) * MIPC1

# cubic coeffs (A=-0.75): w_k = ((d*t + c)*t + b)*t + a
CUBIC = [
    (-0.75, 1.50, -0.75, 0.0),
    (1.25, -2.25, 0.0, 1.0),
    (-1.25, 1.50, 0.75, 0.0),
    (0.75, -0.75, 0.0, 0.0),
]

NG = FL0 // 16                   # 16-col output groups


def _chunks(cap):
    out = []
    o = 0
    while o < cap:
        n = min(MAXCHUNK, cap - o)
        out.append((o, n))
        o += n
    return out


def build_nc(num_devices=NCORES):
    import concourse.bass as bass
    import concourse.tile as tile
    from concourse import bacc, mybir, library_config
    from concourse.masks import make_identity

    f32 = mybir.dt.float32
    f16 = mybir.dt.float16
    i16 = mybir.dt.int16
    i32 = mybir.dt.int32
    Alu = mybir.AluOpType
    Act = mybir.ActivationFunctionType

    nc = bacc.Bacc("TRN2", target_bir_lowering=False, debug=False,
                   num_devices=num_devices)
    vd = nc.dram_tensor("vdT", [3, P, F], f32, kind="ExternalInput").ap()
    sad = nc.dram_tensor("sa", [P, F], f32, kind="ExternalInput").ap()
    ib0d = nc.dram_tensor("ib0", [P, FL0], f32, kind="ExternalInput").ap()
    is0d = nc.dram_tensor("is0", [P, FL0], f32, kind="ExternalInput").ap()
    ib1d = nc.dram_tensor("ib1", [P, 64], f32, kind="ExternalInput").ap()
    exa1 = nc.dram_tensor("exa1", [NPA * NSLOT_A1, 128], f16,
                          kind="ExternalInput").ap()
    exa0 = nc.dram_tensor("exa0", [NPA * NSLOT_A0, 128], f16,
                          kind="ExternalInput").ap()
    exb0 = nc.dram_tensor("exb0", [NPB * NSLOT_B, 128], f16,
                          kind="ExternalInput").ap()
    w1d = nc.dram_tensor("w1j", [P, 4 * P], f16, kind="ExternalInput").ap()
    w2d = nc.dram_tensor("w2q", [P, 3], f16, kind="ExternalInput").ap()
    # selq[h]: [128, 128] selector, S_h[p, m] = 1 iff p == 16*h + m%16
    seld = nc.dram_tensor("selq", [P, 8 * P], f32, kind="ExternalInput").ap()
    out_d = nc.dram_tensor("out", [NG, P, 512], f16, kind="ExternalOutput").ap()

    with tile.TileContext(nc) as tc, ExitStack() as ctx:
        cpool = ctx.enter_context(tc.tile_pool(name="const", bufs=1))
        geom = ctx.enter_context(tc.tile_pool(name="geom", bufs=1))
        gpool = ctx.enter_context(tc.tile_pool(name="gath", bufs=1))
        mpool = ctx.enter_context(tc.tile_pool(name="mlp", bufs=3))
        pp = ctx.enter_context(tc.tile_pool(name="ps", bufs=2, space="PSUM"))
        tpool = ctx.enter_context(tc.tile_pool(name="tmp", bufs=16))
        dve, act = nc.vector, nc.scalar

        nc.gpsimd.load_library(library_config.mlp)

        _tag = [0]

        def t(shape, dt=f32):
            _tag[0] += 1
            return tpool.tile(shape, dt, name=f"g{_tag[0]}", tag="tmp")

        _cb = {}

        def cbias(val):
            if val not in _cb:
                ct = cpool.tile([P, 1], f32, name=f"cb{len(_cb)}",
                                tag=f"cb{len(_cb)}")
                nc.vector.memset(ct[:], float(val))
                _cb[val] = ct
            return _cb[val][:]

        ident = cpool.tile([P, P], f16, name="ident", tag="ident")
        make_identity(nc, ident[:])
        w1_sb = cpool.tile([P, 4 * P], f16, name="w1c", tag="w1")
        nc.sync.dma_start(w1_sb[:], w1d[:, :])
        w2_sb = cpool.tile([P, 3], f16, name="w2c", tag="w2")
        nc.sync.dma_start(w2_sb[:], w2d[:, :])
        sel_sb = cpool.tile([P, 8 * P], f32, name="selc", tag="sel")
        nc.sync.dma_start(sel_sb[:], seld[:, :])

        xt = geom.tile([P, F], f32, name="xt", tag="xt")
        yt = geom.tile([P, F], f32, name="yt", tag="yt")
        zt = geom.tile([P, F], f32, name="zt", tag="zt")
        sat = geom.tile([P, F], f32, name="sat", tag="sat")
        nc.sync.dma_start(xt[:], vd[0])
        nc.sync.dma_start(yt[:], vd[1])
        nc.sync.dma_start(zt[:], vd[2])
        nc.sync.dma_start(sat[:], sad[:, :])
        ib0 = geom.tile([P, FL0], f32, name="ib0", tag="ib0")
        is0 = geom.tile([P, FL0], f32, name="is0", tag="is0")
        ib1 = geom.tile([P, 64], f32, name="ib1", tag="ib1")
        nc.sync.dma_start(ib0[:], ib0d[:, :])
        nc.sync.dma_start(is0[:], is0d[:, :])
        nc.sync.dma_start(ib1[:], ib1d[:, :])

        NL = FL0  # geometry cols

        def X(tile_):
            return tile_[:, 0:NL]

        # ---- gx*pi via half-angle: z>=0: 2*atan(x/(h+|z|));
        #      z<0: sign(x)*pi - 2*atan(x/(h+|z|)),  h = sqrt(1-y^2)
        az = t([P, F]); act.activation(X(az), X(zt), Act.Abs, bias=cbias(0.0))
        y2 = t([P, F]); act.activation(X(y2), X(yt), Act.Square, bias=cbias(0.0))
        hyp = geom.tile([P, F], f32, name="hyp", tag="hyp")
        act.activation(X(hyp), X(y2), Act.Sqrt, bias=cbias(1.0), scale=-1.0)
        den = t([P, F]); dve.tensor_tensor(out=X(den), in0=X(hyp), in1=X(az),
                                           op=Alu.add)
        dve.tensor_scalar(out=X(den), in0=X(den), scalar1=1e-30, scalar2=None,
                          op0=Alu.max)
        rden = t([P, F]); dve.reciprocal(X(rden), X(den))
        rat = t([P, F]); dve.tensor_tensor(out=X(rat), in0=X(xt), in1=X(rden),
                                           op=Alu.mult)
        at = t([P, F]); act.activation(X(at), X(rat), Act.Arctan,
                                       bias=cbias(0.0))
        szlt = t([P, F]); dve.tensor_scalar(out=X(szlt), in0=X(zt), scalar1=0.0,
                                            scalar2=None, op0=Alu.is_lt)
        f2 = t([P, F]); dve.tensor_scalar(out=X(f2), in0=X(szlt), scalar1=-4.0,
                                          scalar2=2.0, op0=Alu.mult,
                                          op1=Alu.add)
        t1 = t([P, F]); dve.tensor_tensor(out=X(t1), in0=X(at), in1=X(f2),
                                          op=Alu.mult)
        sg = t([P, F]); act.activation(X(sg), X(xt), Act.Sign, bias=cbias(0.0))
        t2_ = t([P, F]); dve.tensor_tensor(out=X(t2_), in0=X(szlt), in1=X(sg),
                                           op=Alu.mult)
        gxpi = geom.tile([P, F], f32, name="gxpi", tag="gxpi")
        dve.scalar_tensor_tensor(out=X(gxpi), in0=X(t2_), scalar=PI,
                                 in1=X(t1), op0=Alu.mult, op1=Alu.add)

        # ---- acos(y) = 2*atan(sqrt(1-y^2)/(1+y))
        yc = t([P, F]); dve.tensor_scalar(out=X(yc), in0=X(yt), scalar1=-1.0,
                                          scalar2=1.0, op0=Alu.max, op1=Alu.min)
        dny = t([P, F]); dve.tensor_scalar(out=X(dny), in0=X(yc), scalar1=1.0,
                                           scalar2=1e-30, op0=Alu.add,
                                           op1=Alu.max)
        rdy = t([P, F]); dve.reciprocal(X(rdy), X(dny))
        rty = t([P, F]); dve.tensor_tensor(out=X(rty), in0=X(hyp), in1=X(rdy),
                                           op=Alu.mult)
        acy = geom.tile([P, F], f32, name="acy", tag="acy")
        act.activation(X(acy), X(rty), Act.Arctan, bias=cbias(0.0))

        # ---- mip weights
        lnsa = t([P, F]); act.activation(X(lnsa), X(sat), Act.Ln,
                                         bias=cbias(0.0))
        mipc = t([P, F]); dve.tensor_scalar(out=X(mipc), in0=X(lnsa),
                                            scalar1=MIPC1, scalar2=MIPC2,
                                            op0=Alu.mult, op1=Alu.add)
        dve.tensor_scalar(out=X(mipc), in0=X(mipc), scalar1=1.0, scalar2=3.0,
                          op0=Alu.max, op1=Alu.min)
        t3 = t([P, F]); dve.tensor_scalar(out=X(t3), in0=X(mipc), scalar1=-1.0,
                                          scalar2=3.0, op0=Alu.mult,
                                          op1=Alu.add)
        s0 = geom.tile([P, F], f32, name="s0", tag="s0")
        dve.tensor_scalar(out=X(s0), in0=X(t3), scalar1=1.0, scalar2=None,
                          op0=Alu.min)
        s1 = geom.tile([P, 64], f32, name="s1", tag="s1")
        u1_ = t([P, 64]); dve.tensor_scalar(out=u1_[:], in0=t3[:, 0:64],
                                            scalar1=-1.0, scalar2=0.0,
                                            op0=Alu.add, op1=Alu.max)
        dve.tensor_scalar(out=s1[:], in0=u1_[:], scalar1=1.0, scalar2=0.5,
                          op0=Alu.min, op1=Alu.mult)

        # ---- per-level geometry -> w16 (fp16) and idx (fp32)
        def level_geom(lv, ncols, Wg, Hg, s_ap):
            sl = (slice(None), slice(0, ncols))

            def pt(nm, sh, dt=f32):
                return geom.tile(sh, dt, name=f"{nm}{lv}", tag=f"{nm}{lv}")

            ix4 = pt("ix4", [P, ncols]); act.activation(
                ix4[:], gxpi[sl], Act.Identity,
                bias=cbias((Wg - 1) / 2.0 + 4.0), scale=Wg / (2.0 * PI))
            iy4 = pt("iy4", [P, ncols]); act.activation(
                iy4[:], acy[sl], Act.Identity,
                bias=cbias(3.5), scale=2.0 * Hg / PI)

            def floorf(v, nm):
                vi = t([P, ncols], dt=i32); dve.tensor_copy(out=vi[:], in_=v[:])
                vf = t([P, ncols]); dve.tensor_copy(out=vf[:], in_=vi[:])
                m = t([P, ncols]); dve.tensor_tensor(out=m[:], in0=vf[:],
                                                     in1=v[:], op=Alu.is_gt)
                o = pt(nm, [P, ncols]); dve.scalar_tensor_tensor(
                    out=o[:], in0=m[:], scalar=-1.0, in1=vf[:],
                    op0=Alu.mult, op1=Alu.add)
                return o

            xf = floorf(ix4, "xf")
            yf = floorf(iy4, "yf")
            tx = pt("tx", [P, ncols]); dve.tensor_tensor(
                out=tx[:], in0=ix4[:], in1=xf[:], op=Alu.subtract)
            ty = pt("ty", [P, ncols]); dve.tensor_tensor(
                out=ty[:], in0=iy4[:], in1=yf[:], op=Alu.subtract)
            tys = pt("tys", [P, ncols]); dve.tensor_tensor(
                out=tys[:], in0=ty[:], in1=s_ap, op=Alu.mult)

            wx = pt("wx", [P, ncols * 4])
            wy = pt("wy", [P, ncols * 4])
            for k, (d, c, b, a) in enumerate(CUBIC):
                wxk = bass.AP(wx[:].tensor, wx[:].offset + k,
                              [[ncols * 4, P], [4, ncols]])
                wyk = bass.AP(wy[:].tensor, wy[:].offset + k,
                              [[ncols * 4, P], [4, ncols]])
                u = t([P, ncols]); act.activation(u[:], tx[:], Act.Identity,
                                                  bias=cbias(c), scale=d)
                p2 = t([P, ncols]); dve.tensor_tensor(out=p2[:], in0=u[:],
                                                      in1=tx[:], op=Alu.mult)
                dve.scalar_tensor_tensor(out=wxk, in0=p2[:], scalar=b,
                                         in1=tx[:], op0=Alu.add, op1=Alu.mult)
                uy = t([P, ncols]); act.activation(uy[:], ty[:], Act.Identity,
                                                   bias=cbias(c), scale=d)
                p2y = t([P, ncols]); dve.tensor_tensor(out=p2y[:], in0=uy[:],
                                                       in1=ty[:], op=Alu.mult)
                dve.scalar_tensor_tensor(out=wyk, in0=p2y[:], scalar=b,
                                         in1=tys[:], op0=Alu.add, op1=Alu.mult)
                if k == 1:  # a == 1.0: +1 for wx, +s for wy
                    dve.tensor_scalar(out=wxk, in0=wxk, scalar1=1.0,
                                      scalar2=None, op0=Alu.add)
                    wy1 = t([P, ncols]); dve.tensor_copy(out=wy1[:], in_=wyk)
                    dve.tensor_tensor(out=wyk, in0=wy1[:], in1=s_ap,
                                      op=Alu.add)
            # w16[p, col, k, j] = wx_k * wy_j   (fp16)
            w16 = geom.tile([P, ncols, 16], f16, name=f"w16_{lv}",
                            tag=f"w16_{lv}")
            dve.tensor_tensor(
                out=w16[:, :, :],
                in0=bass.AP(wx[:].tensor, wx[:].offset,
                            [[ncols * 4, P], [4, ncols], [1, 4], [0, 4]]),
                in1=bass.AP(wy[:].tensor, wy[:].offset,
                            [[ncols * 4, P], [4, ncols], [0, 4], [1, 4]]),
                op=Alu.mult)
            return w16, xf, yf

        w16_0, xf0, yf0 = level_geom(0, FL0, W0, H0, s0[:, 0:FL0])
        w16_1, xf1, yf1 = level_geom(1, 64, W1_, H1, s1[:])

        # ---- idx fp32: l0: yf*is0 + xf - ib0 ; l1: yf*31 + xf - ib1
        idxf0 = geom.tile([P, FL0], f32, name="idxf0", tag="idxf0")
        dve.tensor_tensor(out=idxf0[:], in0=yf0[:], in1=is0[:], op=Alu.mult)
        dve.tensor_tensor(out=idxf0[:], in0=idxf0[:], in1=xf0[:], op=Alu.add)
        dve.tensor_tensor(out=idxf0[:], in0=idxf0[:], in1=ib0[:],
                          op=Alu.subtract)
        idxf1 = geom.tile([P, 64], f32, name="idxf1", tag="idxf1")
        dve.scalar_tensor_tensor(out=idxf1[:], in0=yf1[:], scalar=float(SW_A1),
                                 in1=xf1[:], op0=Alu.mult, op1=Alu.add)
        dve.tensor_tensor(out=idxf1[:], in0=idxf1[:], in1=ib1[:],
                          op=Alu.subtract)

        # ---- idx -> wrapped int16 layout: wrap[q, 8*col+h] = idxf[16h+q%16, col]
        # via 8 selector matmuls (replicated across the 8 partition groups)
        def make_wrapped(idxf, ncols, name):
            wrap = geom.tile([P, ncols * 8], i16, name=f"ixW{name}",
                             tag=f"ixW{name}")
            for h in range(8):
                ps = pp.tile([P, ncols], f32, name=f"psW{name}", tag="psW")
                nc.tensor.matmul(out=ps[:], lhsT=sel_sb[:, h * P:(h + 1) * P],
                                 rhs=idxf[:], start=True, stop=True)
                dve.tensor_copy(
                    out=bass.AP(wrap[:].tensor, wrap[:].offset + h,
                                [[ncols * 8, P], [8, ncols]]),
                    in_=ps[:])
            return wrap

        wrap0 = make_wrapped(idxf0, FL0, "0")
        wrap1 = make_wrapped(idxf1, 64, "1")

        # ---- gathers
        g0 = gpool.tile([P, FL0, 128], f16, name="g0", tag="g0")
        g1 = gpool.tile([P, FA, 128], f16, name="g1", tag="g1")

        def gathers(gtile, wrap, caps, table, nslot, colbase):
            col = colbase
            for k, cap in enumerate(caps):
                for (o, n) in _chunks(cap):
                    so = col * 128 + o
                    nc.gpsimd.dma_gather(
                        out_ap=gtile[:, so // 128:(so + n) // 128, :],
                        in_ap=table[k * nslot:(k + 1) * nslot, :],
                        idxs_ap=wrap[:, so // 16:(so + n) // 16],
                        num_idxs=n, num_idxs_reg=n, elem_size=128)
                col += cap // 128

        gathers(g1, wrap1, CAPA, exa1, NSLOT_A1, 0)
        gathers(g0, wrap0, CAPA, exa0, NSLOT_A0, 0)
        gathers(g0, wrap0, CAPB, exb0, NSLOT_B, FA)

        # ---- weighting multiply (fp16 2x), k-tree -> packed emb32, l1 merge
        emb = gpool.tile([P, FL0 * 32], f16, name="emb", tag="emb")

        def wmul(gtile, w16, ncols, wcols, embout):
            gf = gtile[:]
            for k in range(4):
                gk = bass.AP(gf.tensor, gf.offset + k * 32,
                             [[ncols * 128, P], [128, ncols], [1, 32]])
                wk = bass.AP(w16[:].tensor, w16[:].offset + k * 4,
                             [[wcols * 16, P], [16, ncols], [0, 8], [1, 4]])
                dve.tensor_tensor(out=gk, in0=gk, in1=wk, op=Alu.mult)
            a0 = bass.AP(gf.tensor, gf.offset,
                         [[ncols * 128, P], [128, ncols], [1, 64]])
            a1 = bass.AP(gf.tensor, gf.offset + 64,
                         [[ncols * 128, P], [128, ncols], [1, 64]])
            dve.tensor_tensor(out=a0, in0=a0, in1=a1, op=Alu.add)
            b0 = bass.AP(gf.tensor, gf.offset,
                         [[ncols * 128, P], [128, ncols], [1, 32]])
            b1 = bass.AP(gf.tensor, gf.offset + 32,
                         [[ncols * 128, P], [128, ncols], [1, 32]])
            dve.tensor_tensor(out=embout, in0=b0, in1=b1, op=Alu.add)

        e1p = bass.AP(g1[:].tensor, g1[:].offset,
                      [[FA * 128, P], [128, FA], [1, 32]])
        wmul(g1, w16_1, FA, 64, e1p)
        wmul(g0, w16_0, FL0, FL0,
             bass.AP(emb[:].tensor, emb[:].offset,
                     [[FL0 * 32, P], [32, FL0], [1, 32]]))
        # emb[cols 0:FA] += g1.emb32
        e0 = bass.AP(emb[:].tensor, emb[:].offset,
                     [[FL0 * 32, P], [32, FA], [1, 32]])
        dve.tensor_tensor(out=e0, in0=e0, in1=e1p, op=Alu.add)

        # ---- MLP: transpose 4-col tiles, j-fold + W1 via block-diag, relu, W2
        for gg in range(NG):           # 16 cols per group
            ops = pp.tile([P, 512], f32, name="ops", tag="ops")
            for t4 in range(4):        # 4-col tile
                base = gg * 16 + t4 * 4
                pst = pp.tile([P, P], f16, name="pst", tag="pst")
                nc.tensor.transpose(
                    out=pst[:],
                    in_=bass.AP(emb[:].tensor, emb[:].offset + base * 32,
                                [[FL0 * 32, P], [1, 128]]),
                    identity=ident[:])
                rhs = mpool.tile([P, P], f16, name="rhs", tag="rhs")
                act.copy(rhs[:], pst[:])
                hps = pp.tile([P, 512], f32, name="hps", tag="hps")
                for c4 in range(4):
                    nc.tensor.matmul(out=hps[:, c4 * P:(c4 + 1) * P],
                                     lhsT=w1_sb[:, c4 * P:(c4 + 1) * P],
                                     rhs=rhs[:], start=True, stop=True)
                hsb = mpool.tile([P, 512], f16, name="hsb", tag="hsb")
                act.activation(hsb[:], hps[:], Act.Relu, bias=cbias(0.0))
                nc.tensor.matmul(out=ops[32 * t4:32 * t4 + 3, :],
                                 lhsT=w2_sb[:], rhs=hsb[:],
                                 start=True, stop=True,
                                 tile_position=(0, 32 * t4))
            outsb = mpool.tile([P, 512], f16, name="outsb", tag="outsb")
            act.copy(outsb[:], ops[:])
            nc.sync.dma_start(out_d[gg], outsb[:])

    nc.compile()
    return nc


_NC_CACHE = {}


def get_nc(key="full", cfg=None):
    if key not in _NC_CACHE:
        _NC_CACHE[key] = build_nc()
    return _NC_CACHE[key]


FULL_CFG = {}


def _split_lpt(pops, piece, ncores, maxper):
    items = []
    for s, p in enumerate(pops):
        p = int(p)
        while p > 0:
            q = min(p, piece)
            items.append((s, q))
            p -= q
    items.sort(key=lambda x: -x[1])
    loads = [0] * ncores
    bins = [[] for _ in range(ncores)]
    for u, sz in items:
        best = min((c for c in range(ncores) if len(bins[c]) < maxper),
                   key=lambda c: loads[c])
        bins[best].append((u, sz))
        loads[best] += sz
    return bins


def _build_Ey(grid):
    """grid [8, H, W] -> Ey [H+1, W+48, 8, 4] fp16; Ey[R, xp, c, j] =
    grid[c, R-2+j, xp-2], zero outside."""
    C, H, W = grid.shape
    g = np.asarray(grid, np.float16)
    Ey = np.zeros((H + 1, W + 48, C, 4), np.float16)
    gt = np.ascontiguousarray(g.transpose(1, 2, 0))      # [H, W, C]
    for j in range(4):
        rlo = max(0, 2 - j)
        rhi = min(H + 1, H + 2 - j)  # rows with 0 <= R-2+j < H
        Ey[rlo:rhi, 2:W + 2, :, j] = gt[rlo - 2 + j:rhi - 2 + j, :, :]
    return Ey


def _slot_slice(Ey, cbase, width):
    """[(H+1)*width, 128] fp16: slot (R, c) = Ey[R, cbase+c + k] k=0..3,
    layout (k, ch, j)."""
    Hp = Ey.shape[0]
    sl = np.stack([Ey[:, cbase + k:cbase + k + width] for k in range(4)],
                  axis=2)                                # [Hp, w, 4, 8, 4]
    return np.ascontiguousarray(sl).reshape(Hp * width, 128)


def host_prepare(viewdirs, saSample, mats, W1, W2, cfg=None):
    vd = np.asarray(viewdirs, np.float64)
    sa = np.asarray(saSample, np.float64)
    gx = np.arctan2(vd[:, 0], vd[:, 2]) / PI
    mip = np.clip(np.log(sa / saTexel) / (2 * np.log(2.0)), 1.0, 3.0)
    isA = mip < 2.001
    isC = mip >= 3.0
    isB = (~isA) & (~isC)
    C1 = (np.floor(((gx + 1) * W1_ - 1) * 0.5) + 1).astype(np.int64)
    C0 = (np.floor(((gx + 1) * W0 - 1) * 0.5) + 1).astype(np.int64)
    stripA = C1 // SW_A1
    mslabB = C0 // SW_B
    popA = np.bincount(stripA[isA], minlength=(W1_ // SW_A1) + 1)
    popB = np.bincount(mslabB[isB], minlength=(W0 // SW_B) + 1)
    binsA = _split_lpt(popA, PIECE_A, NCORES, NPA)
    binsB = _split_lpt(popB, PIECE_B, NCORES, NPB)

    Ey0 = _build_Ey(mats[0])
    Ey1 = _build_Ey(mats[1])

    # per-strip ray lists (A by strip, B by mslab), order stable
    rayA = {s: np.nonzero(isA & (stripA == s))[0] for s in np.unique(stripA[isA])}
    rayB = {m: np.nonzero(isB & (mslabB == m))[0] for m in np.unique(mslabB[isB])}
    usedA = {s: 0 for s in rayA}
    usedB = {m: 0 for m in rayB}

    in_maps = []
    perms = []
    for c in range(NCORES):
        slots_vd = np.zeros((F * P, 3), np.float32)
        slots_sa = np.full(F * P, 4.09e-5, np.float32)
        ib0v = np.zeros(FL0 * P, np.float32)
        is0v = np.full(FL0 * P, float(SW_B), np.float32)
        ib1v = np.zeros(64 * P, np.float32)
        perm = np.full(F * P, -1, np.int64)

        posA = list(binsA[c]) + [(None, 0)] * (NPA - len(binsA[c]))
        posB = list(binsB[c]) + [(None, 0)] * (NPB - len(binsB[c]))

        exa1c = np.zeros((NPA * NSLOT_A1, 128), np.float16)
        exa0c = np.zeros((NPA * NSLOT_A0, 128), np.float16)
        exb0c = np.zeros((NPB * NSLOT_B, 128), np.float16)

        def pad_ray(Ccenter1):
            # fabricated ray: gx targeting l1 col center, y=0, big sa
            gxi = (2.0 * (Ccenter1 - 0.7) + 1.0) / W1_ - 1.0
            return np.array([np.sin(PI * gxi), 0.0, np.cos(PI * gxi)],
                            np.float32), gxi

        col = 0
        for k in range(NPA):
            s, sz = posA[k]
            cap = CAPA[k]
            if s is None:
                s = posA[0][0] if posA[0][0] is not None else 0
                sz = 0
            rays = rayA.get(s, np.zeros(0, np.int64))
            sel = rays[usedA[s]:usedA[s] + sz]
            usedA[s] += sz
            base = col * P
            n = len(sel)
            perm[base:base + n] = sel
            slots_vd[base:base + n] = vd[sel].astype(np.float32)
            slots_sa[base:base + n] = sa[sel].astype(np.float32)
            pv, pg = pad_ray(SW_A1 * s + SW_A1 // 2)
            slots_vd[base + n:base + cap] = pv
            # l1 idx base: table rows x SW_A1, col base 31*s
            ib1v[base:base + cap] = 3 * SW_A1 + 3 + SW_A1 * s
            # l0-A window
            padC0 = int(np.floor(((pg + 1) * W0 - 1) * 0.5) + 1)
            allC0 = np.concatenate([C0[sel], [padC0]])
            w0b = max(0, min(int(allC0.min()), W0 + 1 - SW_A0))
            if allC0.max() - w0b + 1 > SW_A0:
                raise RuntimeError("A l0 window overflow")
            ib0v[base:base + cap] = 3 * SW_A0 + 3 + w0b
            is0v[base:base + cap] = float(SW_A0)
            exa1c[k * NSLOT_A1:(k + 1) * NSLOT_A1] = _slot_slice(
                Ey1, SW_A1 * s, SW_A1)
            exa0c[k * NSLOT_A0:(k + 1) * NSLOT_A0] = _slot_slice(
                Ey0, w0b, SW_A0)
            col += cap // 128

        assert col == FA
        for k in range(NPB):
            m, sz = posB[k]
            cap = CAPB[k]
            if m is None:
                m = posB[0][0] if posB[0][0] is not None else 0
                sz = 0
            rays = rayB.get(m, np.zeros(0, np.int64))
            sel = rays[usedB[m]:usedB[m] + sz]
            usedB[m] += sz
            base = col * P
            n = len(sel)
            perm[base:base + n] = sel
            slots_vd[base:base + n] = vd[sel].astype(np.float32)
            slots_sa[base:base + n] = sa[sel].astype(np.float32)
            pv, _ = pad_ray(2 * (SW_B * m + SW_B // 2))
            slots_vd[base + n:base + cap] = pv
            ib0v[base:base + cap] = 3 * SW_B + 3 + SW_B * m
            exb0c[k * NSLOT_B:(k + 1) * NSLOT_B] = _slot_slice(
                Ey0, SW_B * m, SW_B)
            col += cap // 128
        assert col == FL0

        # remaining cols: fabricated rays (not computed)
        pv, _ = pad_ray(W1_ // 2)
        slots_vd[FL0 * P:] = pv

        # slot layout is column-major: slot i -> (p, f) = (i%128, i//128)
        vdT = slots_vd.reshape(F, P, 3).transpose(2, 1, 0)
        sam = slots_sa.reshape(F, P).T
        ib1m = np.zeros((64, P), np.float32)
        ib1m[:FA] = ib1v.reshape(64, P)[:FA]
        mm = {"vdT": np.ascontiguousarray(vdT),
              "sa": np.ascontiguousarray(sam),
              "ib0": np.ascontiguousarray(ib0v.reshape(FL0, P).T),
              "is0": np.ascontiguousarray(is0v.reshape(FL0, P).T),
              "ib1": np.ascontiguousarray(ib1m.T),
              "exa1": exa1c, "exa0": exa0c, "exb0": exb0c,
              "selq": _selq(), "w1j": _w1j(W1),
              "w2q": np.ascontiguousarray(np.asarray(W2, np.float16).T)}
        in_maps.append(mm)
        perms.append(perm)
    return in_maps, perms


def _selq():
    s = np.zeros((P, 8, P), np.float32)
    for h in range(8):
        for m in range(P):
            s[16 * h + m % 16, h, m] = 1.0
    return np.ascontiguousarray(s.reshape(P, 8 * P))


def _w1j(W1):
    """[128, 4*128] fp16: block c' rows (c''*32 + cc*4 + j) ->
    W1[f, cc] if c''==c' (summed over j by matmul)."""
    w = np.zeros((128, 4, 128), np.float32)
    W1 = np.asarray(W1, np.float64)
    for cpp in range(4):
        for cc in range(8):
            for j in range(4):
                q = cpp * 32 + cc * 4 + j
                w[q, cpp, :] = W1[:, cc]
    return np.ascontiguousarray(w.reshape(128, 512).astype(np.float16))


def assemble_output(results, perms, cfg=None):
    B = NCORES * 32768
    out = np.zeros((B, 3), np.float32)
    for c, res in enumerate(results):
        o = res["out"].astype(np.float32)     # [NG, 128, 512]
        perm = perms[c]
        # value of slot (p, fcol): fcol = 16*gg + 4*t4 + c4:
        # out_d[gg, 32*t4 + ch, c4*128 + p]
        vals = np.zeros((FL0 * P, 3), np.float32)
        for gg in range(NG):
            for t4 in range(4):
                blk = o[gg, 32 * t4:32 * t4 + 3, :].reshape(3, 4, P)
                for c4 in range(4):
                    fcol = 16 * gg + 4 * t4 + c4
                    vals[fcol * P:(fcol + 1) * P] = blk[:, c4, :].T
        ok = perm[:FL0 * P] >= 0
        out[perm[:FL0 * P][ok]] = vals[ok]
    return out


def kernel(viewdirs, saSample, bg_mat0, bg_mat1, bg_mat2, W1, W2):
    from concourse.bass_utils import run_bass_kernel_spmd
    nc = get_nc()
    in_maps, perms = host_prepare(viewdirs, saSample, [bg_mat0, bg_mat1],
                                  W1, W2)
    res = run_bass_kernel_spmd(nc, in_maps, list(range(NCORES)))
    return assemble_output(res.results, perms)


# revision 27
# speedup vs baseline: 1.1834x; 1.1834x over previous
"""Trainium2 Bass kernel for HierarchicalBG (bicubic pano sampling + MLP).

Strategy (data-parallel over rays, 8 cores):
- Level-2 grid weight is identically 0 (mip clipped to [1,3]) -> never read.
- Rays classified by mip: A (mip<2: needs level0+level1), B (needs l0 only),
  C (mip>=3: output exactly 0, not shipped to device).
- Tables are "footprint-slot" fp16: slot (R,C) = the full 4x4x8 bicubic
  footprint (256B) -> one dma_gather descriptor per (ray, level).
- x-range strip decomposition + LPT bin-packing of strips to cores keeps
  per-instruction idx ranges within int16 for dma_gather; static per-position
  capacities (CAPA/CAPB) with fabricated zero-weight pad rays.
- Device: fp16 weighting multiply (2x DVE mode) + k-tap tree; j-taps and the
  8->128 MLP are fused into one PE matmul per ray column via block-diag
  weights; relu on ACT; 128->3 on PE (col-tiled into one PSUM bank).
"""

import numpy as np
from contextlib import ExitStack

PI = float(np.pi)
P = 128
NCORES = 8

# static per-position ray capacities (slots), multiples of 128
CAPA = [1664, 1152, 1024, 896, 768, 640, 512, 384, 128, 128]   # level-1 strips
CAPB = [4224, 4224, 4096, 3072, 2688, 2176, 1280, 1664]        # l0-only slabs
PIECE_A, PIECE_B = 1536, 4096
FA = sum(CAPA) // 128            # 57 cols
FB = sum(CAPB) // 128            # 183 cols
FL0 = FA + FB                    # 240 cols through l0 + MLP
F = 256                          # total slot cols per core
MAXCHUNK = 1024                  # dma_gather per-instruction idx limit

W0, H0 = 1024, 512
W1_, H1 = 2048, 1024
SW_A1 = 31                       # l1 strip width (cols); idx = R*31+c <= 31774
SW_A0 = 20                       # l0 window width for one l1 strip
SW_B = 21                        # l0 mini-slab width
NSLOT_A1 = (H1 + 1) * SW_A1      # 31775
NSLOT_A0 = (H0 + 1) * SW_A0      # 10260
NSLOT_B = (H0 + 1) * SW_B        # 10773
NPA, NPB = len(CAPA), len(CAPB)

saTexel = 4.0 * PI / (6.0 * 2048 * 2048)
MIPC1 = 1.0 / (2.0 * np.log(2.0))
MIPC2 = -float(np.log(saTexel)) * MIPC1

# cubic coeffs (A=-0.75): w_k = ((d*t + c)*t + b)*t + a
CUBIC = [
    (-0.75, 1.50, -0.75, 0.0),
    (1.25, -2.25, 0.0, 1.0),
    (-1.25, 1.50, 0.75, 0.0),
    (0.75, -0.75, 0.0, 0.0),
]

NG = FL0 // 16                   # 16-col output groups


def _chunks(cap):
    out = []
    o = 0
    while o < cap:
        n = min(MAXCHUNK, cap - o)
        out.append((o, n))
        o += n
    return out


def build_nc(num_devices=NCORES):
    import concourse.bass as bass
    import concourse.tile as tile
    from concourse import bacc, mybir, library_config
    from concourse.masks import make_identity

    f32 = mybir.dt.float32
    f16 = mybir.dt.float16
    i16 = mybir.dt.int16
    i32 = mybir.dt.int32
    Alu = mybir.AluOpType
    Act = mybir.ActivationFunctionType

    nc = bacc.Bacc("TRN2", target_bir_lowering=False, debug=False,
                   num_devices=num_devices)
    vd = nc.dram_tensor("vdT", [3, P, F], f32, kind="ExternalInput").ap()
    sad = nc.dram_tensor("sa", [P, F], f32, kind="ExternalInput").ap()
    ib0d = nc.dram_tensor("ib0", [P, FL0], f32, kind="ExternalInput").ap()
    is0d = nc.dram_tensor("is0", [P, FL0], f32, kind="ExternalInput").ap()
    ib1d = nc.dram_tensor("ib1", [P, 64], f32, kind="ExternalInput").ap()
    exa1 = nc.dram_tensor("exa1", [NPA * NSLOT_A1, 128], f16,
                          kind="ExternalInput").ap()
    exa0 = nc.dram_tensor("exa0", [NPA * NSLOT_A0, 128], f16,
                          kind="ExternalInput").ap()
    exb0 = nc.dram_tensor("exb0", [NPB * NSLOT_B, 128], f16,
                          kind="ExternalInput").ap()
    w1d = nc.dram_tensor("w1j", [P, 4 * P], f16, kind="ExternalInput").ap()
    w2d = nc.dram_tensor("w2q", [P, 3], f16, kind="ExternalInput").ap()
    # selq[h]: [128, 128] selector, S_h[p, m] = 1 iff p == 16*h + m%16
    seld = nc.dram_tensor("selq", [P, 8 * P], f32, kind="ExternalInput").ap()
    out_d = nc.dram_tensor("out", [NG, P, 512], f16, kind="ExternalOutput").ap()

    with tile.TileContext(nc) as tc, ExitStack() as ctx:
        cpool = ctx.enter_context(tc.tile_pool(name="const", bufs=1))
        geom = ctx.enter_context(tc.tile_pool(name="geom", bufs=1))
        gpool = ctx.enter_context(tc.tile_pool(name="gath", bufs=1))
        mpool = ctx.enter_context(tc.tile_pool(name="mlp", bufs=3))
        pp = ctx.enter_context(tc.tile_pool(name="ps", bufs=2, space="PSUM"))
        tpool = ctx.enter_context(tc.tile_pool(name="tmp", bufs=16))
        dve, act = nc.vector, nc.scalar

        nc.gpsimd.load_library(library_config.mlp)

        _tag = [0]

        def t(shape, dt=f32):
            _tag[0] += 1
            return tpool.tile(shape, dt, name=f"g{_tag[0]}", tag="tmp")

        _cb = {}

        def cbias(val):
            if val not in _cb:
                ct = cpool.tile([P, 1], f32, name=f"cb{len(_cb)}",
                                tag=f"cb{len(_cb)}")
                nc.vector.memset(ct[:], float(val))
                _cb[val] = ct
            return _cb[val][:]

        ident = cpool.tile([P, P], f16, name="ident", tag="ident")
        make_identity(nc, ident[:])
        w1_sb = cpool.tile([P, 4 * P], f16, name="w1c", tag="w1")
        nc.sync.dma_start(w1_sb[:], w1d[:, :])
        w2_sb = cpool.tile([P, 3], f16, name="w2c", tag="w2")
        nc.sync.dma_start(w2_sb[:], w2d[:, :])
        sel_sb = cpool.tile([P, 8 * P], f32, name="selc", tag="sel")
        nc.sync.dma_start(sel_sb[:], seld[:, :])

        xt = geom.tile([P, F], f32, name="xt", tag="xt")
        yt = geom.tile([P, F], f32, name="yt", tag="yt")
        zt = geom.tile([P, F], f32, name="zt", tag="zt")
        sat = geom.tile([P, F], f32, name="sat", tag="sat")
        nc.sync.dma_start(xt[:], vd[0])
        nc.sync.dma_start(yt[:], vd[1])
        nc.sync.dma_start(zt[:], vd[2])
        nc.sync.dma_start(sat[:], sad[:, :])
        ib0 = geom.tile([P, FL0], f32, name="ib0", tag="ib0")
        is0 = geom.tile([P, FL0], f32, name="is0", tag="is0")
        ib1 = geom.tile([P, 64], f32, name="ib1", tag="ib1")
        nc.sync.dma_start(ib0[:], ib0d[:, :])
        nc.sync.dma_start(is0[:], is0d[:, :])
        nc.sync.dma_start(ib1[:], ib1d[:, :])

        NL = FL0  # geometry cols

        def X(tile_):
            return tile_[:, 0:NL]

        # ---- gx*pi via half-angle: z>=0: 2*atan(x/(h+|z|));
        #      z<0: sign(x)*pi - 2*atan(x/(h+|z|)),  h = sqrt(1-y^2)
        az = t([P, F]); act.activation(X(az), X(zt), Act.Abs, bias=cbias(0.0))
        y2 = t([P, F]); act.activation(X(y2), X(yt), Act.Square, bias=cbias(0.0))
        hyp = geom.tile([P, F], f32, name="hyp", tag="hyp")
        h0_ = t([P, F]); act.activation(X(h0_), X(y2), Act.Sqrt,
                                        bias=cbias(1.0), scale=-1.0)
        # one Newton step: hyp = (h0 + (1-y^2)/h0) / 2  (ACT sqrt is coarse)
        dve.tensor_scalar(out=X(h0_), in0=X(h0_), scalar1=1e-20, scalar2=None,
                          op0=Alu.max)
        s_ = t([P, F]); dve.tensor_scalar(out=X(s_), in0=X(y2), scalar1=-1.0,
                                          scalar2=1.0, op0=Alu.mult,
                                          op1=Alu.add)
        r_ = t([P, F]); dve.reciprocal(X(r_), X(h0_))
        q_ = t([P, F]); dve.tensor_tensor(out=X(q_), in0=X(s_), in1=X(r_),
                                          op=Alu.mult)
        dve.tensor_tensor(out=X(hyp), in0=X(h0_), in1=X(q_), op=Alu.add)
        dve.tensor_scalar(out=X(hyp), in0=X(hyp), scalar1=0.5, scalar2=None,
                          op0=Alu.mult)
        den = t([P, F]); dve.tensor_tensor(out=X(den), in0=X(hyp), in1=X(az),
                                           op=Alu.add)
        dve.tensor_scalar(out=X(den), in0=X(den), scalar1=1e-30, scalar2=None,
                          op0=Alu.max)
        rden = t([P, F]); dve.reciprocal(X(rden), X(den))
        rat = t([P, F]); dve.tensor_tensor(out=X(rat), in0=X(xt), in1=X(rden),
                                           op=Alu.mult)
        at = t([P, F]); act.activation(X(at), X(rat), Act.Arctan,
                                       bias=cbias(0.0))
        szlt = t([P, F]); dve.tensor_scalar(out=X(szlt), in0=X(zt), scalar1=0.0,
                                            scalar2=None, op0=Alu.is_lt)
        f2 = t([P, F]); dve.tensor_scalar(out=X(f2), in0=X(szlt), scalar1=-4.0,
                                          scalar2=2.0, op0=Alu.mult,
                                          op1=Alu.add)
        t1 = t([P, F]); dve.tensor_tensor(out=X(t1), in0=X(at), in1=X(f2),
                                          op=Alu.mult)
        sg = t([P, F]); act.activation(X(sg), X(xt), Act.Sign, bias=cbias(0.0))
        t2_ = t([P, F]); dve.tensor_tensor(out=X(t2_), in0=X(szlt), in1=X(sg),
                                           op=Alu.mult)
        gxpi = geom.tile([P, F], f32, name="gxpi", tag="gxpi")
        dve.scalar_tensor_tensor(out=X(gxpi), in0=X(t2_), scalar=PI,
                                 in1=X(t1), op0=Alu.mult, op1=Alu.add)

        # ---- acos(y) = 2*atan(sqrt(1-y^2)/(1+y))
        yc = t([P, F]); dve.tensor_scalar(out=X(yc), in0=X(yt), scalar1=-1.0,
                                          scalar2=1.0, op0=Alu.max, op1=Alu.min)
        dny = t([P, F]); dve.tensor_scalar(out=X(dny), in0=X(yc), scalar1=1.0,
                                           scalar2=1e-30, op0=Alu.add,
                                           op1=Alu.max)
        rdy = t([P, F]); dve.reciprocal(X(rdy), X(dny))
        rty = t([P, F]); dve.tensor_tensor(out=X(rty), in0=X(hyp), in1=X(rdy),
                                           op=Alu.mult)
        acy = geom.tile([P, F], f32, name="acy", tag="acy")
        act.activation(X(acy), X(rty), Act.Arctan, bias=cbias(0.0))

        # ---- mip weights
        lnsa = t([P, F]); act.activation(X(lnsa), X(sat), Act.Ln,
                                         bias=cbias(0.0))
        mipc = t([P, F]); dve.tensor_scalar(out=X(mipc), in0=X(lnsa),
                                            scalar1=MIPC1, scalar2=MIPC2,
                                            op0=Alu.mult, op1=Alu.add)
        dve.tensor_scalar(out=X(mipc), in0=X(mipc), scalar1=1.0, scalar2=3.0,
                          op0=Alu.max, op1=Alu.min)
        t3 = t([P, F]); dve.tensor_scalar(out=X(t3), in0=X(mipc), scalar1=-1.0,
                                          scalar2=3.0, op0=Alu.mult,
                                          op1=Alu.add)
        s0 = geom.tile([P, F], f32, name="s0", tag="s0")
        dve.tensor_scalar(out=X(s0), in0=X(t3), scalar1=1.0, scalar2=None,
                          op0=Alu.min)
        s1 = geom.tile([P, 64], f32, name="s1", tag="s1")
        u1_ = t([P, 64]); dve.tensor_scalar(out=u1_[:], in0=t3[:, 0:64],
                                            scalar1=-1.0, scalar2=0.0,
                                            op0=Alu.add, op1=Alu.max)
        dve.tensor_scalar(out=s1[:], in0=u1_[:], scalar1=1.0, scalar2=0.5,
                          op0=Alu.min, op1=Alu.mult)

        # ---- per-level geometry -> w16 (fp16) and idx (fp32)
        def level_floors(lv, ncols, Wg, Hg):
            def pt(nm, sh, dt=f32):
                return geom.tile(sh, dt, name=f"{nm}{lv}", tag=f"{nm}{lv}")

            sl = (slice(None), slice(0, ncols))
            ix4 = pt("ix4", [P, ncols]); act.activation(
                ix4[:], gxpi[sl], Act.Identity,
                bias=cbias((Wg - 1) / 2.0 + 4.0), scale=Wg / (2.0 * PI))
            iy4 = pt("iy4", [P, ncols]); act.activation(
                iy4[:], acy[sl], Act.Identity,
                bias=cbias(3.5), scale=2.0 * Hg / PI)

            def floorf(v, nm):
                vi = t([P, ncols], dt=i32); dve.tensor_copy(out=vi[:], in_=v[:])
                vf = t([P, ncols]); dve.tensor_copy(out=vf[:], in_=vi[:])
                m = t([P, ncols]); dve.tensor_tensor(out=m[:], in0=vf[:],
                                                     in1=v[:], op=Alu.is_gt)
                o = pt(nm, [P, ncols]); dve.scalar_tensor_tensor(
                    out=o[:], in0=m[:], scalar=-1.0, in1=vf[:],
                    op0=Alu.mult, op1=Alu.add)
                return o

            return ix4, iy4, floorf(ix4, "xf"), floorf(iy4, "yf")

        def level_weights(lv, ncols, ix4, iy4, xf, yf, s_ap):
            def pt(nm, sh, dt=f32):
                return geom.tile(sh, dt, name=f"{nm}{lv}", tag=f"{nm}{lv}")

            tx = pt("tx", [P, ncols]); dve.tensor_tensor(
                out=tx[:], in0=ix4[:], in1=xf[:], op=Alu.subtract)
            ty = pt("ty", [P, ncols]); dve.tensor_tensor(
                out=ty[:], in0=iy4[:], in1=yf[:], op=Alu.subtract)
            tys = pt("tys", [P, ncols]); dve.tensor_tensor(
                out=tys[:], in0=ty[:], in1=s_ap, op=Alu.mult)

            wx = pt("wx", [P, ncols * 4])
            wy = pt("wy", [P, ncols * 4])
            for k, (d, c, b, a) in enumerate(CUBIC):
                wxk = bass.AP(wx[:].tensor, wx[:].offset + k,
                              [[ncols * 4, P], [4, ncols]])
                wyk = bass.AP(wy[:].tensor, wy[:].offset + k,
                              [[ncols * 4, P], [4, ncols]])
                u = t([P, ncols]); act.activation(u[:], tx[:], Act.Identity,
                                                  bias=cbias(c), scale=d)
                p2 = t([P, ncols]); dve.tensor_tensor(out=p2[:], in0=u[:],
                                                      in1=tx[:], op=Alu.mult)
                dve.scalar_tensor_tensor(out=wxk, in0=p2[:], scalar=b,
                                         in1=tx[:], op0=Alu.add, op1=Alu.mult)
                uy = t([P, ncols]); act.activation(uy[:], ty[:], Act.Identity,
                                                   bias=cbias(c), scale=d)
                p2y = t([P, ncols]); dve.tensor_tensor(out=p2y[:], in0=uy[:],
                                                       in1=ty[:], op=Alu.mult)
                dve.scalar_tensor_tensor(out=wyk, in0=p2y[:], scalar=b,
                                         in1=tys[:], op0=Alu.add, op1=Alu.mult)
                if k == 1:  # a == 1.0: +1 for wx, +s for wy
                    dve.tensor_scalar(out=wxk, in0=wxk, scalar1=1.0,
                                      scalar2=None, op0=Alu.add)
                    wy1 = t([P, ncols]); dve.tensor_copy(out=wy1[:], in_=wyk)
                    dve.tensor_tensor(out=wyk, in0=wy1[:], in1=s_ap,
                                      op=Alu.add)
            # w16[p, col, k, j] = wx_k * wy_j   (fp16)
            w16 = geom.tile([P, ncols, 16], f16, name=f"w16_{lv}",
                            tag=f"w16_{lv}")
            dve.tensor_tensor(
                out=w16[:, :, :],
                in0=bass.AP(wx[:].tensor, wx[:].offset,
                            [[ncols * 4, P], [4, ncols], [1, 4], [0, 4]]),
                in1=bass.AP(wy[:].tensor, wy[:].offset,
                            [[ncols * 4, P], [4, ncols], [0, 4], [1, 4]]),
                op=Alu.mult)
            return w16

        # ---- floors + idx + gathers FIRST (so gathers start early), cubic
        # weight evaluation overlaps the gather stream.
        ix40, iy40, xf0, yf0 = level_floors(0, FL0, W0, H0)
        ix41, iy41, xf1, yf1 = level_floors(1, 64, W1_, H1)

        idxf0 = geom.tile([P, FL0], f32, name="idxf0", tag="idxf0")
        dve.tensor_tensor(out=idxf0[:], in0=yf0[:], in1=is0[:], op=Alu.mult)
        dve.tensor_tensor(out=idxf0[:], in0=idxf0[:], in1=xf0[:], op=Alu.add)
        dve.tensor_tensor(out=idxf0[:], in0=idxf0[:], in1=ib0[:],
                          op=Alu.subtract)
        idxf1 = geom.tile([P, 64], f32, name="idxf1", tag="idxf1")
        dve.scalar_tensor_tensor(out=idxf1[:], in0=yf1[:], scalar=float(SW_A1),
                                 in1=xf1[:], op0=Alu.mult, op1=Alu.add)
        dve.tensor_tensor(out=idxf1[:], in0=idxf1[:], in1=ib1[:],
                          op=Alu.subtract)

        # idx -> wrapped int16: wrap[q, 8*col+h] = idxf[16h+q%16, col]
        # via 8 selector matmuls (replicated across the 8 partition groups)
        def make_wrapped(idxf, ncols, name):
            wrap = geom.tile([P, ncols * 8], i16, name=f"ixW{name}",
                             tag=f"ixW{name}")
            for h in range(8):
                ps = pp.tile([P, ncols], f32, name=f"psW{name}", tag="psW")
                nc.tensor.matmul(out=ps[:], lhsT=sel_sb[:, h * P:(h + 1) * P],
                                 rhs=idxf[:], start=True, stop=True)
                dve.tensor_copy(
                    out=bass.AP(wrap[:].tensor, wrap[:].offset + h,
                                [[ncols * 8, P], [8, ncols]]),
                    in_=ps[:])
            return wrap

        wrap1 = make_wrapped(idxf1, 64, "1")
        wrap0 = make_wrapped(idxf0, FL0, "0")

        g0 = gpool.tile([P, FL0, 128], f16, name="g0", tag="g0")
        g1 = gpool.tile([P, FA, 128], f16, name="g1", tag="g1")

        def gathers(gtile, wrap, caps, table, nslot, colbase):
            col = colbase
            for k, cap in enumerate(caps):
                for (o, n) in _chunks(cap):
                    so = col * 128 + o
                    nc.gpsimd.dma_gather(
                        out_ap=gtile[:, so // 128:(so + n) // 128, :],
                        in_ap=table[k * nslot:(k + 1) * nslot, :],
                        idxs_ap=wrap[:, so // 16:(so + n) // 16],
                        num_idxs=n, num_idxs_reg=n, elem_size=128)
                col += cap // 128

        gathers(g1, wrap1, CAPA, exa1, NSLOT_A1, 0)
        gathers(g0, wrap0, CAPA, exa0, NSLOT_A0, 0)
        gathers(g0, wrap0, CAPB, exb0, NSLOT_B, FA)

        # cubic weights (overlap with gathers)
        w16_0 = level_weights(0, FL0, ix40, iy40, xf0, yf0, s0[:, 0:FL0])
        w16_1 = level_weights(1, 64, ix41, iy41, xf1, yf1, s1[:])

        # ---- weighting multiply (fp16 2x), k-tree -> packed emb32, l1 merge
        emb = gpool.tile([P, FL0 * 32], f16, name="emb", tag="emb")

        def wmul(gtile, w16, ncols, wcols, embt, estride, CH=48):
            gf = gtile[:]
            for c0 in range(0, ncols, CH):
                n = min(CH, ncols - c0)
                for k in range(4):
                    gk = bass.AP(gf.tensor, gf.offset + c0 * 128 + k * 32,
                                 [[ncols * 128, P], [128, n], [1, 32]])
                    wk = bass.AP(w16[:].tensor,
                                 w16[:].offset + c0 * 16 + k * 4,
                                 [[wcols * 16, P], [16, n], [0, 8], [1, 4]])
                    dve.tensor_tensor(out=gk, in0=gk, in1=wk, op=Alu.mult)
                a0 = bass.AP(gf.tensor, gf.offset + c0 * 128,
                             [[ncols * 128, P], [128, n], [1, 64]])
                a1 = bass.AP(gf.tensor, gf.offset + c0 * 128 + 64,
                             [[ncols * 128, P], [128, n], [1, 64]])
                dve.tensor_tensor(out=a0, in0=a0, in1=a1, op=Alu.add)
                b0 = bass.AP(gf.tensor, gf.offset + c0 * 128,
                             [[ncols * 128, P], [128, n], [1, 32]])
                b1 = bass.AP(gf.tensor, gf.offset + c0 * 128 + 32,
                             [[ncols * 128, P], [128, n], [1, 32]])
                eo = bass.AP(embt[:].tensor, embt[:].offset + c0 * estride,
                             [[ncols * estride, P], [estride, n], [1, 32]])
                dve.tensor_tensor(out=eo, in0=b0, in1=b1, op=Alu.add)

        wmul(g1, w16_1, FA, 64, g1, 128)
        wmul(g0, w16_0, FL0, FL0, emb, 32)
        # emb[cols 0:FA] += g1.emb32  (two chunks for finer deps)
        for c0, n in ((0, 32), (32, FA - 32)):
            e0 = bass.AP(emb[:].tensor, emb[:].offset + c0 * 32,
                         [[FL0 * 32, P], [32, n], [1, 32]])
            e1 = bass.AP(g1[:].tensor, g1[:].offset + c0 * 128,
                         [[FA * 128, P], [128, n], [1, 32]])
            dve.tensor_tensor(out=e0, in0=e0, in1=e1, op=Alu.add)

        # ---- MLP: transpose 4-col tiles, j-fold + W1 via block-diag, relu, W2
        for gg in range(NG):           # 16 cols per group
            ops = pp.tile([P, 512], f32, name="ops", tag="ops")
            for t4 in range(4):        # 4-col tile
                base = gg * 16 + t4 * 4
                pst = pp.tile([P, P], f16, name="pst", tag="pst")
                nc.tensor.transpose(
                    out=pst[:],
                    in_=bass.AP(emb[:].tensor, emb[:].offset + base * 32,
                                [[FL0 * 32, P], [1, 128]]),
                    identity=ident[:])
                rhs = mpool.tile([P, P], f16, name="rhs", tag="rhs")
                act.copy(rhs[:], pst[:])
                hps = pp.tile([P, 512], f32, name="hps", tag="hps")
                for c4 in range(4):
                    nc.tensor.matmul(out=hps[:, c4 * P:(c4 + 1) * P],
                                     lhsT=w1_sb[:, c4 * P:(c4 + 1) * P],
                                     rhs=rhs[:], start=True, stop=True)
                hsb = mpool.tile([P, 512], f16, name="hsb", tag="hsb")
                act.activation(hsb[:], hps[:], Act.Relu, bias=cbias(0.0))
                nc.tensor.matmul(out=ops[32 * t4:32 * t4 + 3, :],
                                 lhsT=w2_sb[:], rhs=hsb[:],
                                 start=True, stop=True,
                                 tile_position=(0, 32 * t4))
            outsb = mpool.tile([P, 512], f16, name="outsb", tag="outsb")
            act.copy(outsb[:], ops[:])
            nc.sync.dma_start(out_d[gg], outsb[:])

    nc.compile()
    return nc


_NC_CACHE = {}


def get_nc(key="full", cfg=None):
    if key not in _NC_CACHE:
        _NC_CACHE[key] = build_nc()
    return _NC_CACHE[key]


FULL_CFG = {}


def _split_lpt(pops, piece, ncores, maxper):
    items = []
    for s, p in enumerate(pops):
        p = int(p)
        while p > 0:
            q = min(p, piece)
            items.append((s, q))
            p -= q
    items.sort(key=lambda x: -x[1])
    loads = [0] * ncores
    bins = [[] for _ in range(ncores)]
    for u, sz in items:
        best = min((c for c in range(ncores) if len(bins[c]) < maxper),
                   key=lambda c: loads[c])
        bins[best].append((u, sz))
        loads[best] += sz
    return bins


def _build_Ey(grid):
    """grid [8, H, W] -> Ey [H+1, W+48, 8, 4] fp16; Ey[R, xp, c, j] =
    grid[c, R-2+j, xp-2], zero outside."""
    C, H, W = grid.shape
    g = np.asarray(grid, np.float16)
    Ey = np.zeros((H + 1, W + 48, C, 4), np.float16)
    gt = np.ascontiguousarray(g.transpose(1, 2, 0))      # [H, W, C]
    for j in range(4):
        rlo = max(0, 2 - j)
        rhi = min(H + 1, H + 2 - j)  # rows with 0 <= R-2+j < H
        Ey[rlo:rhi, 2:W + 2, :, j] = gt[rlo - 2 + j:rhi - 2 + j, :, :]
    return Ey


def _slot_slice(Ey, cbase, width):
    """[(H+1)*width, 128] fp16: slot (R, c) = Ey[R, cbase+c + k] k=0..3,
    layout (k, ch, j)."""
    Hp = Ey.shape[0]
    sl = np.stack([Ey[:, cbase + k:cbase + k + width] for k in range(4)],
                  axis=2)                                # [Hp, w, 4, 8, 4]
    return np.ascontiguousarray(sl).reshape(Hp * width, 128)


def host_prepare(viewdirs, saSample, mats, W1, W2, cfg=None):
    vd = np.asarray(viewdirs, np.float64)
    sa = np.asarray(saSample, np.float64)
    gx = np.arctan2(vd[:, 0], vd[:, 2]) / PI
    mip = np.clip(np.log(sa / saTexel) / (2 * np.log(2.0)), 1.0, 3.0)
    isA = mip < 2.001
    isC = mip >= 3.0
    isB = (~isA) & (~isC)
    C1 = (np.floor(((gx + 1) * W1_ - 1) * 0.5) + 1).astype(np.int64)
    C0 = (np.floor(((gx + 1) * W0 - 1) * 0.5) + 1).astype(np.int64)
    stripA = C1 // SW_A1
    mslabB = C0 // SW_B
    popA = np.bincount(stripA[isA], minlength=(W1_ // SW_A1) + 1)
    popB = np.bincount(mslabB[isB], minlength=(W0 // SW_B) + 1)
    binsA = _split_lpt(popA, PIECE_A, NCORES, NPA)
    binsB = _split_lpt(popB, PIECE_B, NCORES, NPB)

    Ey0 = _build_Ey(mats[0])
    Ey1 = _build_Ey(mats[1])

    # per-strip ray lists (A by strip, B by mslab), order stable
    rayA = {s: np.nonzero(isA & (stripA == s))[0] for s in np.unique(stripA[isA])}
    rayB = {m: np.nonzero(isB & (mslabB == m))[0] for m in np.unique(mslabB[isB])}
    usedA = {s: 0 for s in rayA}
    usedB = {m: 0 for m in rayB}

    in_maps = []
    perms = []
    for c in range(NCORES):
        slots_vd = np.zeros((F * P, 3), np.float32)
        slots_sa = np.full(F * P, 4.09e-5, np.float32)
        ib0v = np.zeros(FL0 * P, np.float32)
        is0v = np.full(FL0 * P, float(SW_B), np.float32)
        ib1v = np.zeros(64 * P, np.float32)
        perm = np.full(F * P, -1, np.int64)

        posA = list(binsA[c]) + [(None, 0)] * (NPA - len(binsA[c]))
        posB = list(binsB[c]) + [(None, 0)] * (NPB - len(binsB[c]))

        exa1c = np.zeros((NPA * NSLOT_A1, 128), np.float16)
        exa0c = np.zeros((NPA * NSLOT_A0, 128), np.float16)
        exb0c = np.zeros((NPB * NSLOT_B, 128), np.float16)

        def pad_ray(Ccenter1):
            # fabricated ray: gx targeting l1 col center, y=0, big sa
            gxi = (2.0 * (Ccenter1 - 0.7) + 1.0) / W1_ - 1.0
            return np.array([np.sin(PI * gxi), 0.0, np.cos(PI * gxi)],
                            np.float32), gxi

        col = 0
        for k in range(NPA):
            s, sz = posA[k]
            cap = CAPA[k]
            if s is None:
                s = posA[0][0] if posA[0][0] is not None else 0
                sz = 0
            rays = rayA.get(s, np.zeros(0, np.int64))
            sel = rays[usedA[s]:usedA[s] + sz]
            usedA[s] += sz
            base = col * P
            n = len(sel)
            perm[base:base + n] = sel
            slots_vd[base:base + n] = vd[sel].astype(np.float32)
            slots_sa[base:base + n] = sa[sel].astype(np.float32)
            pv, pg = pad_ray(SW_A1 * s + SW_A1 // 2)
            slots_vd[base + n:base + cap] = pv
            # l1 idx base: table rows x SW_A1, col base 31*s
            ib1v[base:base + cap] = 3 * SW_A1 + 3 + SW_A1 * s
            # l0-A window
            padC0 = int(np.floor(((pg + 1) * W0 - 1) * 0.5) + 1)
            allC0 = np.concatenate([C0[sel], [padC0]])
            w0b = max(0, min(int(allC0.min()), W0 + 1 - SW_A0))
            if allC0.max() - w0b + 1 > SW_A0:
                raise RuntimeError("A l0 window overflow")
            ib0v[base:base + cap] = 3 * SW_A0 + 3 + w0b
            is0v[base:base + cap] = float(SW_A0)
            exa1c[k * NSLOT_A1:(k + 1) * NSLOT_A1] = _slot_slice(
                Ey1, SW_A1 * s, SW_A1)
            exa0c[k * NSLOT_A0:(k + 1) * NSLOT_A0] = _slot_slice(
                Ey0, w0b, SW_A0)
            col += cap // 128

        assert col == FA
        for k in range(NPB):
            m, sz = posB[k]
            cap = CAPB[k]
            if m is None:
                m = posB[0][0] if posB[0][0] is not None else 0
                sz = 0
            rays = rayB.get(m, np.zeros(0, np.int64))
            sel = rays[usedB[m]:usedB[m] + sz]
            usedB[m] += sz
            base = col * P
            n = len(sel)
            perm[base:base + n] = sel
            slots_vd[base:base + n] = vd[sel].astype(np.float32)
            slots_sa[base:base + n] = sa[sel].astype(np.float32)
            pv, _ = pad_ray(2 * (SW_B * m + SW_B // 2))
            slots_vd[base + n:base + cap] = pv
            ib0v[base:base + cap] = 3 * SW_B + 3 + SW_B * m
            exb0c[k * NSLOT_B:(k + 1) * NSLOT_B] = _slot_slice(
                Ey0, SW_B * m, SW_B)
            col += cap // 128
        assert col == FL0

        # remaining cols: fabricated rays (not computed)
        pv, _ = pad_ray(W1_ // 2)
        slots_vd[FL0 * P:] = pv

        # slot layout is column-major: slot i -> (p, f) = (i%128, i//128)
        vdT = slots_vd.reshape(F, P, 3).transpose(2, 1, 0)
        sam = slots_sa.reshape(F, P).T
        ib1m = np.zeros((64, P), np.float32)
        ib1m[:FA] = ib1v.reshape(64, P)[:FA]
        mm = {"vdT": np.ascontiguousarray(vdT),
              "sa": np.ascontiguousarray(sam),
              "ib0": np.ascontiguousarray(ib0v.reshape(FL0, P).T),
              "is0": np.ascontiguousarray(is0v.reshape(FL0, P).T),
              "ib1": np.ascontiguousarray(ib1m.T),
              "exa1": exa1c, "exa0": exa0c, "exb0": exb0c,
              "selq": _selq(), "w1j": _w1j(W1),
              "w2q": np.ascontiguousarray(np.asarray(W2, np.float16).T)}
        in_maps.append(mm)
        perms.append(perm)
    return in_maps, perms


def _selq():
    s = np.zeros((P, 8, P), np.float32)
    for h in range(8):
        for m in range(P):
            s[16 * h + m % 16, h, m] = 1.0
    return np.ascontiguousarray(s.reshape(P, 8 * P))


def _w1j(W1):
    """[128, 4*128] fp16: block c' rows (c''*32 + cc*4 + j) ->
    W1[f, cc] if c''==c' (summed over j by matmul)."""
    w = np.zeros((128, 4, 128), np.float32)
    W1 = np.asarray(W1, np.float64)
    for cpp in range(4):
        for cc in range(8):
            for j in range(4):
                q = cpp * 32 + cc * 4 + j
                w[q, cpp, :] = W1[:, cc]
    return np.ascontiguousarray(w.reshape(128, 512).astype(np.float16))


def assemble_output(results, perms, cfg=None):
    B = NCORES * 32768
    out = np.zeros((B, 3), np.float32)
    for c, res in enumerate(results):
        o = res["out"].astype(np.float32)     # [NG, 128, 512]
        perm = perms[c]
        # value of slot (p, fcol): fcol = 16*gg + 4*t4 + c4:
        # out_d[gg, 32*t4 + ch, c4*128 + p]
        vals = np.zeros((FL0 * P, 3), np.float32)
        for gg in range(NG):
            for t4 in range(4):
                blk = o[gg, 32 * t4:32 * t4 + 3, :].reshape(3, 4, P)
                for c4 in range(4):
                    fcol = 16 * gg + 4 * t4 + c4
                    vals[fcol * P:(fcol + 1) * P] = blk[:, c4, :].T
        ok = perm[:FL0 * P] >= 0
        out[perm[:FL0 * P][ok]] = vals[ok]
    return out


def kernel(viewdirs, saSample, bg_mat0, bg_mat1, bg_mat2, W1, W2):
    from concourse.bass_utils import run_bass_kernel_spmd
    nc = get_nc()
    in_maps, perms = host_prepare(viewdirs, saSample, [bg_mat0, bg_mat1],
                                  W1, W2)
    res = run_bass_kernel_spmd(nc, in_maps, list(range(NCORES)))
    return assemble_output(res.results, perms)
